# revision 8
# baseline (speedup 1.0000x reference)
"""Trainium2 Bass kernel for a softmax-free cross-attention block.

Math (per batch b):
  q  = dec @ Wq.T + bq                       [T, D]
  k  = enc @ Wk.T + bk ; v = enc @ Wv.T + bv [T, D]
  kv[h] = k_h.T @ v_h                        [dh, dh]  (contraction over T_enc)
  att   = scale * q_h @ kv[h]                [T, D]
  out   = LayerNorm(att + dec) * gamma + beta

Key algebraic restructuring (all O(n^3) work stays on device):
  kv[h] = Wk_h (enc.T enc) Wv_h.T  -- Gram matrix G replaces the K/V
  projections (one G serves all heads / both K and V), and
  att = dec @ WeffT with WeffT[:, h] = Wq_h.T kv[h] -- a single dense
  matmul replaces Q-projection + per-head attention apply.

Sharding: 8 cores = 4 batches x 2 decoder-halves. Each core computes G for
its full batch (duplicated within the pair -- no collectives) and the output
rows for its 1024 decoder tokens.

Bias handling: bq is applied exactly on-device (rank-1 matmul into the att
accumulation); bk/bv enter kv only through rank-1 correction terms which are
precomputed on host (O(D^2) work; exactly zero for the given inputs).

Matmuls run as float32r (tf32-like, ~1e-4 relative error), accumulation fp32.
"""

import numpy as np

import concourse.bass as bass
import concourse.mybir as mybir
import concourse.tile as tile
from concourse import bacc, bass_utils

D = 1024
H = 16
DH = 64
T_ENC = 2048  # encoder tokens per batch (full batch per core)
TC = 1024  # decoder tokens per core
NT = D // 128  # 8 tiles of 128 along any D-sized dim
NTE = T_ENC // 128  # 16 encoder token tiles
B = 4
T = 2048
SCALE = 1.0 / np.sqrt(DH)
LN_EPS = 1e-5

_CACHE = {}


def _body(tc, nc, d, t_enc):
    f32 = mybir.dt.float32
    f32r = mybir.dt.float32r
    AF = mybir.ActivationFunctionType
    OP = mybir.AluOpType
    nte = t_enc // 128

    ap = tc.alloc_tile_pool  # shorthand

    small = ap(name="small", bufs=1)
    psum_big = ap(name="psum_big", bufs=2, space="PSUM")
    psum_kv = ap(name="psum_kv", bufs=1, space="PSUM")
    ep = ap(name="ep", bufs=3)
    ep1 = ap(name="ep1", bufs=4)

    kv_sb = small.tile([DH, D], f32r, tag="kv")
    attb_sb = small.tile([1, D], f32r, tag="attb")
    bq_sb = small.tile([DH, H], f32r, tag="bq")
    ones_sb = small.tile([1, 128], f32r, tag="ones")
    kvc_sb = small.tile([DH, D], f32, tag="kvc")
    gam_sb = small.tile([128, D], f32, tag="gam")
    bet_sb = small.tile([128, D], f32, tag="bet")
    eps_sb = small.tile([128, 1], f32, tag="eps")

    nc.sync.dma_start(out=bq_sb, in_=d["bqh"])
    nc.sync.dma_start(out=ones_sb, in_=d["ones"])
    nc.sync.dma_start(out=kvc_sb, in_=d["kvc"])
    gam = d["gamma"]
    bet = d["beta"]
    nc.sync.dma_start(
        out=gam_sb,
        in_=bass.AP(tensor=gam.tensor, offset=gam.offset, ap=[[0, 128]] + gam.ap),
    )
    nc.sync.dma_start(
        out=bet_sb,
        in_=bass.AP(tensor=bet.tensor, offset=bet.offset, ap=[[0, 128]] + bet.ap),
    )
    nc.vector.memset(eps_sb, LN_EPS)

    # ---- Phase G: G = enc.T @ enc  [D, D] ----------------------------------
    g_pool = ap(name="gp", bufs=1)
    g_sb = g_pool.tile([128, NT, D], f32r, tag="g")
    enc_pool = ap(name="encp", bufs=1)
    enc_sb = enc_pool.tile([128, nte, D], f32r, tag="enc")
    for t in range(nte):
        nc.sync.dma_start(out=enc_sb[:, t, :], in_=d["enc"][t * 128:(t + 1) * 128, :])
    for i in range(NT):
        pg = psum_big.tile([128, D], f32, tag="pbig")
        for t in range(nte):
            st, sp = t == 0, t == nte - 1
            lhs = enc_sb[:, t, i * 128:(i + 1) * 128]
            nc.tensor.matmul(pg[:, 0:512], lhs, enc_sb[:, t, 0:512], start=st, stop=sp)
            nc.tensor.matmul(pg[:, 512:1024], lhs, enc_sb[:, t, 512:1024], start=st, stop=sp)
        nc.scalar.copy(out=g_sb[:, i, :], in_=pg[:, :])
    enc_pool.release()

    # ---- Phase T1t: T1t = G @ Wk.T  ( = (Wk G).T )  [D(j), D(hd)] ----------
    t1_pool = ap(name="t1p", bufs=1)
    t1_sb = t1_pool.tile([128, NT, D], f32r, tag="t1")
    wk_pool = ap(name="wkp", bufs=1)
    wk_sb = wk_pool.tile([128, NT, D], f32r, tag="wk")
    for i in range(NT):
        nc.sync.dma_start(out=wk_sb[:, i, :], in_=d["wkt"][i * 128:(i + 1) * 128, :])
    for j in range(NT):
        pt = psum_big.tile([128, D], f32, tag="pbig")
        for i in range(NT):
            st, sp = i == 0, i == NT - 1
            lhs = g_sb[:, i, j * 128:(j + 1) * 128]
            nc.tensor.matmul(pt[:, 0:512], lhs, wk_sb[:, i, 0:512], start=st, stop=sp)
            nc.tensor.matmul(pt[:, 512:1024], lhs, wk_sb[:, i, 512:1024], start=st, stop=sp)
        nc.scalar.copy(out=t1_sb[:, j, :], in_=pt[:, :])
    wk_pool.release()

    # ---- Phase kv: kv[h] = T1t_h.T @ WvT_h  [dh, H*dh] ---------------------
    wv_pool = ap(name="wvp", bufs=1)
    wv_sb = wv_pool.tile([128, NT, D], f32r, tag="wv")
    for j in range(NT):
        nc.sync.dma_start(out=wv_sb[:, j, :], in_=d["wvt"][j * 128:(j + 1) * 128, :])
    pkv = psum_kv.tile([DH, D], f32, tag="pkv")
    for h in range(H):
        hs = slice(h * DH, (h + 1) * DH)
        for j in range(NT):
            st = h % 8 == 0 and j == 0
            sp = h % 8 == 7 and j == NT - 1
            nc.tensor.matmul(pkv[:, hs], t1_sb[:, j, hs], wv_sb[:, j, hs], start=st, stop=sp)
    # kv += host-precomputed bias corrections (rank-1 terms; zeros when biases are zero)
    nc.vector.tensor_add(out=kv_sb, in0=pkv[:, :], in1=kvc_sb)
    wv_pool.release()
    t1_pool.release()
    g_pool.release()

    # ---- attb = bq_h.T @ kv[h] (row vector added to every att row) ---------
    pab = psum_kv.tile([1, D], f32, tag="pab")
    for h in range(H):
        hs = slice(h * DH, (h + 1) * DH)
        st, sp = h % 8 == 0, h % 8 == 7
        nc.tensor.matmul(pab[:, hs], bq_sb[:, h:h + 1], kv_sb[:, hs], start=st, stop=sp)
    nc.scalar.copy(out=attb_sb, in_=pab[:, :])

    # ---- Phase Weff: WeffT[i, he] = sum_d Wq[hd, i] kv[h][d, e] ------------
    weff_pool = ap(name="weffp", bufs=1)
    weff_sb = weff_pool.tile([128, NT, D], f32r, tag="weff")
    wq_pool = ap(name="wqp", bufs=1)
    wq_sb = wq_pool.tile([DH, H, D], f32r, tag="wqh")
    for h in range(H):
        nc.sync.dma_start(out=wq_sb[:, h, :], in_=d["wq"][h * DH:(h + 1) * DH, :])
    for i in range(NT):
        pw = psum_big.tile([128, D], f32, tag="pbig")
        for h in range(H):
            hs = slice(h * DH, (h + 1) * DH)
            st, sp = h % 8 == 0, h % 8 == 7
            lhs = wq_sb[:, h, i * 128:(i + 1) * 128]
            nc.tensor.matmul(pw[:, hs], lhs, kv_sb[:, hs], start=st, stop=sp)
        nc.scalar.copy(out=weff_sb[:, i, :], in_=pw[:, :])
    wq_pool.release()

    # ---- Phase att + residual + LayerNorm ----------------------------------
    dt_pool = ap(name="dtp", bufs=1)
    dt_sb = dt_pool.tile([128, NT, TC], f32r, tag="dect")
    for i in range(NT):
        nc.sync.dma_start(out=dt_sb[:, i, :], in_=d["dect"][i * 128:(i + 1) * 128, :])
    for t in range(TC // 128):
        pa = psum_big.tile([128, D], f32, tag="pbig")
        for i in range(NT):
            st = i == 0
            lhs = dt_sb[:, i, t * 128:(t + 1) * 128]
            nc.tensor.matmul(pa[:, 0:512], lhs, weff_sb[:, i, 0:512], start=st, stop=False)
            nc.tensor.matmul(pa[:, 512:1024], lhs, weff_sb[:, i, 512:1024], start=st, stop=False)
        # q-bias contribution: broadcast attb row into every token row
        nc.tensor.matmul(pa[:, 0:512], ones_sb, attb_sb[:, 0:512], start=False, stop=True)
        nc.tensor.matmul(pa[:, 512:1024], ones_sb, attb_sb[:, 512:1024], start=False, stop=True)

        x = ep.tile([128, D], f32, tag="x")
        nc.scalar.activation(out=x, in_=pa[:, :], func=mybir.ActivationFunctionType.Copy, scale=SCALE)
        dct = ep.tile([128, D], f32, tag="dc")
        nc.sync.dma_start(out=dct, in_=d["dec"][t * 128:(t + 1) * 128, :])
        nc.vector.tensor_add(out=x, in0=x, in1=dct)
        stats = ep1.tile([128, 2, 6], f32, tag="st")
        nc.vector.bn_stats(out=stats[:, 0, :], in_=x[:, 0:512])
        nc.vector.bn_stats(out=stats[:, 1, :], in_=x[:, 512:1024])
        mv = ep1.tile([128, 2], f32, tag="mv")
        nc.vector.bn_aggr(out=mv, in_=stats)
        rstd = ep1.tile([128, 1], f32, tag="rs")
        nc.scalar.activation(out=rstd, in_=mv[:, 1:2], func=AF.Sqrt, bias=eps_sb, scale=1.0)
        nc.vector.reciprocal(out=rstd, in_=rstd)
        nc.vector.tensor_scalar(
            out=x, in0=x, scalar1=mv[:, 0:1], scalar2=rstd,
            op0=OP.subtract, op1=OP.mult,
        )
        nc.vector.tensor_mul(out=x, in0=x, in1=gam_sb)
        nc.vector.tensor_add(out=x, in0=x, in1=bet_sb)
        nc.sync.dma_start(out=d["out"][t * 128:(t + 1) * 128, :], in_=x)
    dt_pool.release()
    weff_pool.release()

    ep1.release()
    ep.release()
    psum_kv.release()
    psum_big.release()
    small.release()


def build_program(t_enc=T_ENC):
    nc = bacc.Bacc(
        "TRN2",
        target_bir_lowering=False,
        debug=False,
        enable_asserts=False,
        num_devices=8,
    )
    f32 = mybir.dt.float32
    f32r = mybir.dt.float32r
    specs = {
        "enc": ([t_enc, D], f32r, "ExternalInput"),
        "dect": ([D, TC], f32r, "ExternalInput"),
        "dec": ([TC, D], f32, "ExternalInput"),
        "wq": ([D, D], f32r, "ExternalInput"),
        "wkt": ([D, D], f32r, "ExternalInput"),
        "wvt": ([D, D], f32r, "ExternalInput"),
        "bqh": ([DH, H], f32r, "ExternalInput"),
        "ones": ([1, 128], f32r, "ExternalInput"),
        "kvc": ([DH, D], f32, "ExternalInput"),
        "gamma": ([D], f32, "ExternalInput"),
        "beta": ([D], f32, "ExternalInput"),
        "out": ([TC, D], f32, "ExternalOutput"),
    }
    d = {
        name: nc.dram_tensor(name, shape, dt, kind=kind).ap()
        for name, (shape, dt, kind) in specs.items()
    }

    with tile.TileContext(nc) as tc:
        _body(tc, nc, d, t_enc)
    nc.compile()
    return nc


def make_in_maps(inputs):
    """Shard full inputs into 8 per-core input maps."""
    enc_f = np.ascontiguousarray(np.asarray(inputs["encoder_seq"], dtype=np.float32))
    dec_f = np.ascontiguousarray(np.asarray(inputs["decoder_seq"], dtype=np.float32))
    wq = np.ascontiguousarray(np.asarray(inputs["Wq"], dtype=np.float32))
    wk = np.asarray(inputs["Wk"], dtype=np.float32)
    wv = np.asarray(inputs["Wv"], dtype=np.float32)
    bq = np.asarray(inputs["bq"], dtype=np.float32)
    bk = np.asarray(inputs["bk"], dtype=np.float32)
    bv = np.asarray(inputs["bv"], dtype=np.float32)
    gamma = np.ascontiguousarray(np.asarray(inputs["ln_gamma"], dtype=np.float32))
    beta = np.ascontiguousarray(np.asarray(inputs["ln_beta"], dtype=np.float32))

    wkt = np.ascontiguousarray(wk.T)
    wvt = np.ascontiguousarray(wv.T)
    bqh = np.ascontiguousarray(bq.reshape(H, DH).T)  # [DH, H]
    ones = np.ones((1, 128), dtype=np.float32)

    in_maps = []
    for c in range(8):
        b, p = divmod(c, 2)
        enc_b = np.ascontiguousarray(enc_f[b])  # [T_ENC, D]
        dec_h = np.ascontiguousarray(dec_f[b, p * TC:(p + 1) * TC])  # [TC, D]
        dect = np.ascontiguousarray(dec_h.T)  # [D, TC]

        # kv bias corrections (rank-1; exactly zero for zero biases):
        # kv_full[h] = k_h.T v_h with k = enc Wk.T + bk, v = enc Wv.T + bv
        #   = Wk_h G Wv_h.T + bk_h (x) sv_h + sk_h (x) bv_h + T bk_h (x) bv_h
        # where s = sum_t enc[t], sk_h = Wk_h s, sv_h = Wv_h s.
        kvc = np.zeros((DH, D), dtype=np.float32)
        if bk.any() or bv.any():
            s = enc_b.sum(axis=0)  # [D]
            sk = (wk @ s).reshape(H, DH)
            sv = (wv @ s).reshape(H, DH)
            bkh = bk.reshape(H, DH)
            bvh = bv.reshape(H, DH)
            for h in range(H):
                corr = (
                    np.outer(bkh[h], sv[h])
                    + np.outer(sk[h], bvh[h])
                    + T_ENC * np.outer(bkh[h], bvh[h])
                )
                kvc[:, h * DH:(h + 1) * DH] = corr
        in_maps.append({
            "enc": enc_b,
            "dect": dect,
            "dec": dec_h,
            "wq": wq,
            "wkt": wkt,
            "wvt": wvt,
            "bqh": bqh,
            "ones": ones,
            "kvc": kvc,
            "gamma": gamma,
            "beta": beta,
        })
    return in_maps


def kernel(**inputs):
    if "nc" not in _CACHE:
        _CACHE["nc"] = build_program()
    nc = _CACHE["nc"]
    in_maps = make_in_maps(inputs)
    res = bass_utils.run_bass_kernel_spmd(nc, in_maps, core_ids=list(range(8)))
    out = np.empty((B, T, D), dtype=np.float32)
    for c in range(8):
        b, p = divmod(c, 2)
        out[b, p * TC:(p + 1) * TC] = res.results[c]["out"]
    return out


# revision 20
# speedup vs baseline: 1.2841x; 1.2841x over previous
"""Trainium2 Bass kernel for a softmax-free cross-attention block.

Math (per batch b):
  q  = dec @ Wq.T + bq                       [T, D]
  k  = enc @ Wk.T + bk ; v = enc @ Wv.T + bv [T, D]
  kv[h] = k_h.T @ v_h                        [dh, dh]  (contraction over T_enc)
  att   = scale * q_h @ kv[h]                [T, D]
  out   = LayerNorm(att + dec) * gamma + beta

Key algebraic restructuring (all O(n^3) work stays on device):
  kv[h] = Wk_h (enc.T enc) Wv_h.T  -- Gram matrix G replaces the K/V
  projections (one G serves all heads / both K and V), and
  att = dec @ WeffT with WeffT[:, h] = Wq_h.T kv[h] -- a single dense
  matmul replaces Q-projection + per-head attention apply.

Sharding: 8 cores = 4 batches x 2 decoder-halves. Each core computes G for
its full batch (duplicated within the pair -- no collectives) and the output
rows for its 1024 decoder tokens.

Bias handling: bq is applied exactly on-device (rank-1 matmul into the att
accumulation); bk/bv enter kv only through rank-1 correction terms which are
precomputed on host (O(D^2) work; exactly zero for the given inputs).

Matmuls run as float32r (tf32-like, ~1e-4 relative error), accumulation fp32.
"""

import numpy as np

import concourse.bass as bass
import concourse.mybir as mybir
import concourse.tile as tile
from concourse import bacc, bass_utils

D = 1024
H = 16
DH = 64
T_ENC = 2048  # encoder tokens per batch (full batch per core)
TC = 1024  # decoder tokens per core
NT = D // 128  # 8 tiles of 128 along any D-sized dim
NTE = T_ENC // 128  # 16 encoder token tiles
B = 4
T = 2048
SCALE = 1.0 / np.sqrt(DH)
LN_EPS = 1e-5

_CACHE = {}


def _body(tc, nc, d, t_enc):
    f32 = mybir.dt.float32
    f32r = mybir.dt.float32r
    AF = mybir.ActivationFunctionType
    OP = mybir.AluOpType
    nte = t_enc // 128

    ap = tc.alloc_tile_pool  # shorthand

    small = ap(name="small", bufs=1, side="left")
    psum_big = ap(name="psum_big", bufs=2, space="PSUM")
    psum_kv = ap(name="psum_kv", bufs=1, space="PSUM")
    ep = ap(name="ep", bufs=3, side="left")
    ep1 = ap(name="ep1", bufs=4, side="left")

    # kv [64, D] computed on partitions 0-63, then duplicated to 64-127 so the
    # Weff stage can pair it with natural-Wq row slices at base partition 0/64
    kv_sb = small.tile([128, D], f32r, tag="kv")
    attb_sb = small.tile([1, D], f32r, tag="attb")
    bq_sb = small.tile([DH, H], f32r, tag="bq")
    ones_sb = small.tile([1, 128], f32r, tag="ones")
    kvc_sb = small.tile([DH, D], f32, tag="kvc")
    gam_sb = small.tile([128, D], f32, tag="gam")
    bet_sb = small.tile([128, D], f32, tag="bet")
    eps_sb = small.tile([128, 1], f32, tag="eps")

    nc.sync.dma_start(out=bq_sb, in_=d["bqh"])
    nc.sync.dma_start(out=ones_sb, in_=d["ones"])
    nc.sync.dma_start(out=kvc_sb, in_=d["kvc"])
    gam = d["gamma"]
    bet = d["beta"]
    nc.sync.dma_start(
        out=gam_sb,
        in_=bass.AP(tensor=gam.tensor, offset=gam.offset, ap=[[0, 128]] + gam.ap),
    )
    nc.sync.dma_start(
        out=bet_sb,
        in_=bass.AP(tensor=bet.tensor, offset=bet.offset, ap=[[0, 128]] + bet.ap),
    )
    nc.vector.memset(eps_sb, LN_EPS)

    # ---- Phase G: G = enc.T @ enc  [D, D] ----------------------------------
    # Pool plan (left/right stacks so weight loads prefetch into fresh space):
    #  left:  g(32K) t1(32K) | wq(32K after enc freed)
    #  right: wk(32K) enc(64K->freed) | wv(32K) | weff(32K) dect(32K)
    g_pool = ap(name="gp", bufs=1, side="left")
    g_sb = g_pool.tile([128, NT, D], f32r, tag="g")
    t1_pool = ap(name="t1p", bufs=1, side="left")
    t1_sb = t1_pool.tile([128, NT, D], f32r, tag="t1")
    wk_pool = ap(name="wkp", bufs=1, side="right")
    wk_sb = wk_pool.tile([128, NT, D], f32r, tag="wk")
    enc_pool = ap(name="encp", bufs=1, side="right")
    enc_sb = enc_pool.tile([128, nte, D], f32r, tag="enc")
    for t in range(nte):
        nc.sync.dma_start(out=enc_sb[:, t, :], in_=d["enc"][t * 128:(t + 1) * 128, :])
    for i in range(NT):
        nc.sync.dma_start(out=wk_sb[:, i, :], in_=d["wkt"][i * 128:(i + 1) * 128, :])
    for i in range(NT):
        pg = psum_big.tile([128, D], f32, tag="pbig")
        for t in range(nte):
            st, sp = t == 0, t == nte - 1
            lhs = enc_sb[:, t, i * 128:(i + 1) * 128]
            nc.tensor.matmul(pg[:, 0:512], lhs, enc_sb[:, t, 0:512], start=st, stop=sp)
            nc.tensor.matmul(pg[:, 512:1024], lhs, enc_sb[:, t, 512:1024], start=st, stop=sp)
        nc.scalar.copy(out=g_sb[:, i, :], in_=pg[:, :])
    enc_pool.release()

    # wq into enc's old space (loads overlap T1t), wv into fresh right space
    wq_pool = ap(name="wqp", bufs=1, side="left")
    wq_sb = wq_pool.tile([128, NT, D], f32r, tag="wqn")
    for r in range(NT):
        nc.sync.dma_start(out=wq_sb[:, r, :], in_=d["wq"][r * 128:(r + 1) * 128, :])
    wv_pool = ap(name="wvp", bufs=1, side="right")
    wv_sb = wv_pool.tile([128, NT, D], f32r, tag="wv")
    for j in range(NT):
        nc.sync.dma_start(out=wv_sb[:, j, :], in_=d["wvt"][j * 128:(j + 1) * 128, :])

    # ---- Phase T1t: T1t = G @ Wk.T  ( = (Wk G).T )  [D(j), D(hd)] ----------
    for j in range(NT):
        pt = psum_big.tile([128, D], f32, tag="pbig")
        for i in range(NT):
            st, sp = i == 0, i == NT - 1
            lhs = g_sb[:, i, j * 128:(j + 1) * 128]
            nc.tensor.matmul(pt[:, 0:512], lhs, wk_sb[:, i, 0:512], start=st, stop=sp)
            nc.tensor.matmul(pt[:, 512:1024], lhs, wk_sb[:, i, 512:1024], start=st, stop=sp)
        nc.scalar.copy(out=t1_sb[:, j, :], in_=pt[:, :])

    # ---- Phase kv: kv[h] = T1t_h.T @ WvT_h  [64, D] on partitions 0-63 -----
    pkv = psum_kv.tile([DH, D], f32, tag="pkv")
    for h in range(H):
        hs = slice(h * DH, (h + 1) * DH)
        for j in range(NT):
            st = h % 8 == 0 and j == 0
            sp = h % 8 == 7 and j == NT - 1
            nc.tensor.matmul(pkv[:, hs], t1_sb[:, j, hs], wv_sb[:, j, hs], start=st, stop=sp)
    # kv += host-precomputed bias corrections (rank-1 terms; zeros when biases are zero)
    nc.vector.tensor_add(out=kv_sb[0:DH, :], in0=pkv[:, :], in1=kvc_sb)
    # duplicate to partitions 64-127 for the Weff stage's odd-head row groups
    nc.sync.dma_start(out=kv_sb[DH:2 * DH, :], in_=kv_sb[0:DH, :])
    wv_pool.release()
    wk_pool.release()

    # ---- attb = bq_h.T @ kv[h] (row vector added to every att row) ---------
    pab = psum_kv.tile([1, D], f32, tag="pab")
    for h in range(H):
        hs = slice(h * DH, (h + 1) * DH)
        st, sp = h % 8 == 0, h % 8 == 7
        nc.tensor.matmul(
            pab[:, hs], bq_sb[:, h:h + 1], kv_sb[0:DH, hs], start=st, stop=sp,
        )
    nc.scalar.copy(out=attb_sb, in_=pab[:, :])

    # ---- Phase Weff: WeffT[i, he] = sum_d Wq[hd, i] kv[h][d, e] ------------
    # consecutive heads alternate PE row-groups (partition offset 0/64) so their
    # weight loads and matmuls overlap in the array
    weff_pool = ap(name="weffp", bufs=1, side="right")
    weff_sb = weff_pool.tile([128, NT, D], f32r, tag="weff")
    dt_pool = ap(name="dtp", bufs=1, side="right")
    dt_sb = dt_pool.tile([128, NT, TC], f32r, tag="dect")
    for i in range(NT):
        nc.sync.dma_start(out=dt_sb[:, i, :], in_=d["dect"][i * 128:(i + 1) * 128, :])
    for i in range(NT):
        pw = psum_big.tile([128, D], f32, tag="pbig")
        # group heads by partition base: interleaving base-0/base-64 fp32r
        # matmuls per-MM hangs the fp32 weight-load path on HW
        order = [h for h in range(H) if h % 2 == 0] + [h for h in range(H) if h % 2 == 1]
        for idx, h in enumerate(order):
            hs = slice(h * DH, (h + 1) * DH)
            po = (h % 2) * DH
            bank = h // 8
            st = all(order[k] // 8 != bank for k in range(idx))
            sp = all(order[k] // 8 != bank for k in range(idx + 1, H))
            lhs = wq_sb[po:po + DH, h // 2, i * 128:(i + 1) * 128]
            nc.tensor.matmul(pw[:, hs], lhs, kv_sb[po:po + DH, hs], start=st, stop=sp)
        nc.scalar.copy(out=weff_sb[:, i, :], in_=pw[:, :])

    # ---- Phase att + residual + LayerNorm ----------------------------------
    for t in range(TC // 128):
        pa = psum_big.tile([128, D], f32, tag="pbig")
        for i in range(NT):
            st = i == 0
            lhs = dt_sb[:, i, t * 128:(t + 1) * 128]
            nc.tensor.matmul(pa[:, 0:512], lhs, weff_sb[:, i, 0:512], start=st, stop=False)
            nc.tensor.matmul(pa[:, 512:1024], lhs, weff_sb[:, i, 512:1024], start=st, stop=False)
        # q-bias contribution: broadcast attb row into every token row
        nc.tensor.matmul(pa[:, 0:512], ones_sb, attb_sb[:, 0:512], start=False, stop=True)
        nc.tensor.matmul(pa[:, 512:1024], ones_sb, attb_sb[:, 512:1024], start=False, stop=True)

        x = ep.tile([128, D], f32, tag="x")
        nc.scalar.activation(out=x, in_=pa[:, :], func=mybir.ActivationFunctionType.Copy, scale=SCALE)
        dct = ep.tile([128, D], f32, tag="dc")
        nc.sync.dma_start(out=dct, in_=d["dec"][t * 128:(t + 1) * 128, :])
        nc.vector.tensor_add(out=x, in0=x, in1=dct)
        stats = ep1.tile([128, 2, 6], f32, tag="st")
        nc.vector.bn_stats(out=stats[:, 0, :], in_=x[:, 0:512])
        nc.vector.bn_stats(out=stats[:, 1, :], in_=x[:, 512:1024])
        mv = ep1.tile([128, 2], f32, tag="mv")
        nc.vector.bn_aggr(out=mv, in_=stats)
        rstd = ep1.tile([128, 1], f32, tag="rs")
        nc.scalar.activation(out=rstd, in_=mv[:, 1:2], func=AF.Sqrt, bias=eps_sb, scale=1.0)
        nc.vector.reciprocal(out=rstd, in_=rstd)
        nc.vector.tensor_scalar(
            out=x, in0=x, scalar1=mv[:, 0:1], scalar2=rstd,
            op0=OP.subtract, op1=OP.mult,
        )
        nc.vector.tensor_mul(out=x, in0=x, in1=gam_sb)
        nc.vector.tensor_add(out=x, in0=x, in1=bet_sb)
        nc.sync.dma_start(out=d["out"][t * 128:(t + 1) * 128, :], in_=x)
    dt_pool.release()
    weff_pool.release()
    wq_pool.release()
    t1_pool.release()
    g_pool.release()

    ep1.release()
    ep.release()
    psum_kv.release()
    psum_big.release()
    small.release()


def build_program(t_enc=T_ENC):
    nc = bacc.Bacc(
        "TRN2",
        target_bir_lowering=False,
        debug=False,
        enable_asserts=False,
        num_devices=8,
    )
    f32 = mybir.dt.float32
    f32r = mybir.dt.float32r
    specs = {
        "enc": ([t_enc, D], f32r, "ExternalInput"),
        "dect": ([D, TC], f32r, "ExternalInput"),
        "dec": ([TC, D], f32, "ExternalInput"),
        "wq": ([D, D], f32r, "ExternalInput"),
        "wkt": ([D, D], f32r, "ExternalInput"),
        "wvt": ([D, D], f32r, "ExternalInput"),
        "bqh": ([DH, H], f32r, "ExternalInput"),
        "ones": ([1, 128], f32r, "ExternalInput"),
        "kvc": ([DH, D], f32, "ExternalInput"),
        "gamma": ([D], f32, "ExternalInput"),
        "beta": ([D], f32, "ExternalInput"),
        "out": ([TC, D], f32, "ExternalOutput"),
    }
    d = {
        name: nc.dram_tensor(name, shape, dt, kind=kind).ap()
        for name, (shape, dt, kind) in specs.items()
    }

    with tile.TileContext(nc) as tc:
        _body(tc, nc, d, t_enc)
    nc.compile()
    return nc


def make_in_maps(inputs):
    """Shard full inputs into 8 per-core input maps."""
    enc_f = np.ascontiguousarray(np.asarray(inputs["encoder_seq"], dtype=np.float32))
    dec_f = np.ascontiguousarray(np.asarray(inputs["decoder_seq"], dtype=np.float32))
    wq = np.ascontiguousarray(np.asarray(inputs["Wq"], dtype=np.float32))
    wk = np.asarray(inputs["Wk"], dtype=np.float32)
    wv = np.asarray(inputs["Wv"], dtype=np.float32)
    bq = np.asarray(inputs["bq"], dtype=np.float32)
    bk = np.asarray(inputs["bk"], dtype=np.float32)
    bv = np.asarray(inputs["bv"], dtype=np.float32)
    gamma = np.ascontiguousarray(np.asarray(inputs["ln_gamma"], dtype=np.float32))
    beta = np.ascontiguousarray(np.asarray(inputs["ln_beta"], dtype=np.float32))

    wkt = np.ascontiguousarray(wk.T)
    wvt = np.ascontiguousarray(wv.T)
    bqh = np.ascontiguousarray(bq.reshape(H, DH).T)  # [DH, H]
    ones = np.ones((1, 128), dtype=np.float32)

    in_maps = []
    for c in range(8):
        b, p = divmod(c, 2)
        enc_b = np.ascontiguousarray(enc_f[b])  # [T_ENC, D]
        dec_h = np.ascontiguousarray(dec_f[b, p * TC:(p + 1) * TC])  # [TC, D]
        dect = np.ascontiguousarray(dec_h.T)  # [D, TC]

        # kv bias corrections (rank-1; exactly zero for zero biases):
        # kv_full[h] = k_h.T v_h with k = enc Wk.T + bk, v = enc Wv.T + bv
        #   = Wk_h G Wv_h.T + bk_h (x) sv_h + sk_h (x) bv_h + T bk_h (x) bv_h
        # where s = sum_t enc[t], sk_h = Wk_h s, sv_h = Wv_h s.
        kvc = np.zeros((DH, D), dtype=np.float32)
        if bk.any() or bv.any():
            s = enc_b.sum(axis=0)  # [D]
            sk = (wk @ s).reshape(H, DH)
            sv = (wv @ s).reshape(H, DH)
            bkh = bk.reshape(H, DH)
            bvh = bv.reshape(H, DH)
            for h in range(H):
                corr = (
                    np.outer(bkh[h], sv[h])
                    + np.outer(sk[h], bvh[h])
                    + T_ENC * np.outer(bkh[h], bvh[h])
                )
                kvc[:, h * DH:(h + 1) * DH] = corr
        in_maps.append({
            "enc": enc_b,
            "dect": dect,
            "dec": dec_h,
            "wq": wq,
            "wkt": wkt,
            "wvt": wvt,
            "bqh": bqh,
            "ones": ones,
            "kvc": kvc,
            "gamma": gamma,
            "beta": beta,
        })
    return in_maps


def kernel(**inputs):
    if "nc" not in _CACHE:
        _CACHE["nc"] = build_program()
    nc = _CACHE["nc"]
    in_maps = make_in_maps(inputs)
    res = bass_utils.run_bass_kernel_spmd(nc, in_maps, core_ids=list(range(8)))
    out = np.empty((B, T, D), dtype=np.float32)
    for c in range(8):
        b, p = divmod(c, 2)
        out[b, p * TC:(p + 1) * TC] = res.results[c]["out"]
    return out


# revision 24
# speedup vs baseline: 1.3394x; 1.0431x over previous
"""Trainium2 Bass kernel for a softmax-free cross-attention block.

Math (per batch b):
  q  = dec @ Wq.T + bq                       [T, D]
  k  = enc @ Wk.T + bk ; v = enc @ Wv.T + bv [T, D]
  kv[h] = k_h.T @ v_h                        [dh, dh]  (contraction over T_enc)
  att   = scale * q_h @ kv[h]                [T, D]
  out   = LayerNorm(att + dec) * gamma + beta

Key algebraic restructuring (all O(n^3) work stays on device):
  kv[h] = Wk_h (enc.T enc) Wv_h.T  -- Gram matrix G replaces the K/V
  projections (one G serves all heads / both K and V), and
  att = dec @ WeffT with WeffT[:, h] = Wq_h.T kv[h] -- a single dense
  matmul replaces Q-projection + per-head attention apply.

Sharding: 8 cores = 4 batches x 2 decoder-halves. Each core computes G for
its full batch (duplicated within the pair -- no collectives) and the output
rows for its 1024 decoder tokens.

Bias handling: bq is applied exactly on-device (rank-1 matmul into the att
accumulation); bk/bv enter kv only through rank-1 correction terms which are
precomputed on host (O(D^2) work; exactly zero for the given inputs).

Matmuls run as float32r (tf32-like, ~1e-4 relative error), accumulation fp32.
"""

import numpy as np

import concourse.bass as bass
import concourse.mybir as mybir
import concourse.tile as tile
from concourse import bacc, bass_utils

D = 1024
H = 16
DH = 64
T_ENC = 2048  # encoder tokens per batch (full batch per core)
TC = 1024  # decoder tokens per core
NT = D // 128  # 8 tiles of 128 along any D-sized dim
NTE = T_ENC // 128  # 16 encoder token tiles
B = 4
T = 2048
SCALE = 1.0 / np.sqrt(DH)
LN_EPS = 1e-5

_CACHE = {}


def _body(tc, nc, d, t_enc):
    f32 = mybir.dt.float32
    f32r = mybir.dt.float32r
    AF = mybir.ActivationFunctionType
    OP = mybir.AluOpType
    nte = t_enc // 128

    ap = tc.alloc_tile_pool  # shorthand

    small = ap(name="small", bufs=1, side="left")
    psum_big = ap(name="psum_big", bufs=3, space="PSUM")
    psum_kv = ap(name="psum_kv", bufs=1, space="PSUM")
    ep = ap(name="ep", bufs=3, side="left")
    ep1 = ap(name="ep1", bufs=4, side="left")

    # kv [64, D] computed on partitions 0-63, then duplicated to 64-127 so the
    # Weff stage can pair it with natural-Wq row slices at base partition 0/64
    kv_sb = small.tile([128, D], f32r, tag="kv")
    attb_sb = small.tile([1, D], f32r, tag="attb")
    bq_sb = small.tile([DH, H], f32r, tag="bq")
    ones_sb = small.tile([1, 128], f32r, tag="ones")
    kvc_sb = small.tile([DH, D], f32, tag="kvc")
    gam_sb = small.tile([128, D], f32, tag="gam")
    bet_sb = small.tile([128, D], f32, tag="bet")
    eps_sb = small.tile([128, 1], f32, tag="eps")

    nc.sync.dma_start(out=bq_sb, in_=d["bqh"])
    nc.sync.dma_start(out=ones_sb, in_=d["ones"])
    nc.sync.dma_start(out=kvc_sb, in_=d["kvc"])
    gam = d["gamma"]
    bet = d["beta"]
    nc.sync.dma_start(
        out=gam_sb,
        in_=bass.AP(tensor=gam.tensor, offset=gam.offset, ap=[[0, 128]] + gam.ap),
    )
    nc.sync.dma_start(
        out=bet_sb,
        in_=bass.AP(tensor=bet.tensor, offset=bet.offset, ap=[[0, 128]] + bet.ap),
    )
    nc.vector.memset(eps_sb, LN_EPS)

    # ---- Phase G: G = enc.T @ enc  [D, D] ----------------------------------
    # Pool plan (left/right stacks so weight loads prefetch into fresh space):
    #  left:  g(32K) t1(32K) | wq(32K after enc freed)
    #  right: wk(32K) enc(64K->freed) | wv(32K) | weff(32K) dect(32K)
    g_pool = ap(name="gp", bufs=1, side="left")
    g_sb = g_pool.tile([128, NT, D], f32r, tag="g")
    t1_pool = ap(name="t1p", bufs=1, side="left")
    t1_sb = t1_pool.tile([128, NT, D], f32r, tag="t1")
    wk_pool = ap(name="wkp", bufs=1, side="right")
    wk_sb = wk_pool.tile([128, NT, D], f32r, tag="wk")
    enc_pool = ap(name="encp", bufs=1, side="right")
    enc_sb = enc_pool.tile([128, nte, D], f32r, tag="enc")
    for t in range(nte):
        nc.sync.dma_start(out=enc_sb[:, t, :], in_=d["enc"][t * 128:(t + 1) * 128, :])
    for i in range(NT):
        nc.sync.dma_start(out=wk_sb[:, i, :], in_=d["wkt"][i * 128:(i + 1) * 128, :])
    # two sweeps over encoder halves so matmuls start after ~half the encoder
    # DMA has landed (psum can't hold 8 i-tiles at once, SBUF accumulates)
    half = nte // 2
    for sweep in range(2):
        for i in range(NT):
            pg = psum_big.tile([128, D], f32, tag="pbig")
            for tt in range(half):
                t = sweep * half + tt
                st, sp = tt == 0, tt == half - 1
                lhs = enc_sb[:, t, i * 128:(i + 1) * 128]
                nc.tensor.matmul(pg[:, 0:512], lhs, enc_sb[:, t, 0:512], start=st, stop=sp)
                nc.tensor.matmul(pg[:, 512:1024], lhs, enc_sb[:, t, 512:1024], start=st, stop=sp)
            if sweep == 0:
                nc.scalar.copy(out=g_sb[:, i, :], in_=pg[:, :])
            else:
                nc.vector.tensor_add(out=g_sb[:, i, :], in0=g_sb[:, i, :], in1=pg[:, :])
    enc_pool.release()

    # wq into enc's old space (loads overlap T1t), wv into fresh right space
    wq_pool = ap(name="wqp", bufs=1, side="left")
    wq_sb = wq_pool.tile([128, NT, D], f32r, tag="wqn")
    for r in range(NT):
        nc.sync.dma_start(out=wq_sb[:, r, :], in_=d["wq"][r * 128:(r + 1) * 128, :])
    wv_pool = ap(name="wvp", bufs=1, side="right")
    wv_sb = wv_pool.tile([128, NT, D], f32r, tag="wv")
    for j in range(NT):
        nc.sync.dma_start(out=wv_sb[:, j, :], in_=d["wvt"][j * 128:(j + 1) * 128, :])

    # ---- Phase T1t: T1t = G @ Wk.T  ( = (Wk G).T )  [D(j), D(hd)] ----------
    for j in range(NT):
        pt = psum_big.tile([128, D], f32, tag="pbig")
        for i in range(NT):
            st, sp = i == 0, i == NT - 1
            lhs = g_sb[:, i, j * 128:(j + 1) * 128]
            nc.tensor.matmul(pt[:, 0:512], lhs, wk_sb[:, i, 0:512], start=st, stop=sp)
            nc.tensor.matmul(pt[:, 512:1024], lhs, wk_sb[:, i, 512:1024], start=st, stop=sp)
        nc.scalar.copy(out=t1_sb[:, j, :], in_=pt[:, :])

    # ---- Phase kv: kv[h] = T1t_h.T @ WvT_h  [64, D] on partitions 0-63 -----
    pkv = psum_kv.tile([DH, D], f32, tag="pkv")
    for h in range(H):
        hs = slice(h * DH, (h + 1) * DH)
        for j in range(NT):
            st = h % 8 == 0 and j == 0
            sp = h % 8 == 7 and j == NT - 1
            nc.tensor.matmul(pkv[:, hs], t1_sb[:, j, hs], wv_sb[:, j, hs], start=st, stop=sp)
    # fold the attention scale into kv, then add host-precomputed bias
    # corrections (pre-scaled on host; zeros when biases are zero)
    nc.vector.tensor_scalar(
        out=kv_sb[0:DH, :], in0=pkv[:, :], scalar1=SCALE, scalar2=None, op0=OP.mult,
    )
    nc.vector.tensor_add(out=kv_sb[0:DH, :], in0=kv_sb[0:DH, :], in1=kvc_sb)
    # duplicate to partitions 64-127 for the Weff stage's odd-head row groups
    nc.sync.dma_start(out=kv_sb[DH:2 * DH, :], in_=kv_sb[0:DH, :])
    wv_pool.release()
    wk_pool.release()

    # ---- attb = bq_h.T @ kv[h] (row vector added to every att row) ---------
    pab = psum_kv.tile([DH, D], f32, tag="pkv")
    for h in range(H):
        hs = slice(h * DH, (h + 1) * DH)
        st, sp = h % 8 == 0, h % 8 == 7
        nc.tensor.matmul(
            pab[0:1, hs], bq_sb[:, h:h + 1], kv_sb[0:DH, hs], start=st, stop=sp,
        )
    nc.scalar.copy(out=attb_sb, in_=pab[0:1, :])

    # ---- Phase Weff: WeffT[i, he] = sum_d Wq[hd, i] kv[h][d, e] ------------
    # consecutive heads alternate PE row-groups (partition offset 0/64) so their
    # weight loads and matmuls overlap in the array
    weff_pool = ap(name="weffp", bufs=1, side="right")
    weff_sb = weff_pool.tile([128, NT, D], f32r, tag="weff")
    dt_pool = ap(name="dtp", bufs=1, side="right")
    dt_sb = dt_pool.tile([128, NT, TC], f32r, tag="dect")
    for i in range(NT):
        nc.sync.dma_start(out=dt_sb[:, i, :], in_=d["dect"][i * 128:(i + 1) * 128, :])
    for i in range(NT):
        pw = psum_big.tile([128, D], f32, tag="pbig")
        # group heads by partition base: interleaving base-0/base-64 fp32r
        # matmuls per-MM hangs the fp32 weight-load path on HW
        order = [h for h in range(H) if h % 2 == 0] + [h for h in range(H) if h % 2 == 1]
        for idx, h in enumerate(order):
            hs = slice(h * DH, (h + 1) * DH)
            po = (h % 2) * DH
            bank = h // 8
            st = all(order[k] // 8 != bank for k in range(idx))
            sp = all(order[k] // 8 != bank for k in range(idx + 1, H))
            lhs = wq_sb[po:po + DH, h // 2, i * 128:(i + 1) * 128]
            nc.tensor.matmul(pw[:, hs], lhs, kv_sb[po:po + DH, hs], start=st, stop=sp)
        nc.scalar.copy(out=weff_sb[:, i, :], in_=pw[:, :])

    # ---- Phase att + residual + LayerNorm ----------------------------------
    for t in range(TC // 128):
        pa = psum_big.tile([128, D], f32, tag="pbig")
        dct = ep.tile([128, D], f32, tag="dc")
        nc.sync.dma_start(out=dct, in_=d["dec"][t * 128:(t + 1) * 128, :])
        for i in range(NT):
            st = i == 0
            lhs = dt_sb[:, i, t * 128:(t + 1) * 128]
            nc.tensor.matmul(pa[:, 0:512], lhs, weff_sb[:, i, 0:512], start=st, stop=False)
            nc.tensor.matmul(pa[:, 512:1024], lhs, weff_sb[:, i, 512:1024], start=st, stop=False)
        # q-bias contribution: broadcast attb row into every token row
        nc.tensor.matmul(pa[:, 0:512], ones_sb, attb_sb[:, 0:512], start=False, stop=True)
        nc.tensor.matmul(pa[:, 512:1024], ones_sb, attb_sb[:, 512:1024], start=False, stop=True)

        # residual add straight from PSUM (scale already folded into kv)
        x = ep.tile([128, D], f32, tag="x")
        nc.vector.tensor_add(out=x, in0=pa[:, :], in1=dct)
        stats = ep1.tile([128, 2, 6], f32, tag="st")
        nc.vector.bn_stats(out=stats[:, 0, :], in_=x[:, 0:512])
        nc.vector.bn_stats(out=stats[:, 1, :], in_=x[:, 512:1024])
        mv = ep1.tile([128, 2], f32, tag="mv")
        nc.vector.bn_aggr(out=mv, in_=stats)
        rstd = ep1.tile([128, 1], f32, tag="rs")
        nc.scalar.activation(out=rstd, in_=mv[:, 1:2], func=AF.Sqrt, bias=eps_sb, scale=1.0)
        nc.vector.reciprocal(out=rstd, in_=rstd)
        nc.vector.tensor_scalar(
            out=x, in0=x, scalar1=mv[:, 0:1], scalar2=rstd,
            op0=OP.subtract, op1=OP.mult,
        )
        nc.vector.tensor_mul(out=x, in0=x, in1=gam_sb)
        nc.gpsimd.tensor_add(out=x, in0=x, in1=bet_sb)
        nc.sync.dma_start(out=d["out"][t * 128:(t + 1) * 128, :], in_=x)
    dt_pool.release()
    weff_pool.release()
    wq_pool.release()
    t1_pool.release()
    g_pool.release()

    ep1.release()
    ep.release()
    psum_kv.release()
    psum_big.release()
    small.release()


def build_program(t_enc=T_ENC):
    nc = bacc.Bacc(
        "TRN2",
        target_bir_lowering=False,
        debug=False,
        enable_asserts=False,
        num_devices=8,
    )
    f32 = mybir.dt.float32
    f32r = mybir.dt.float32r
    specs = {
        "enc": ([t_enc, D], f32r, "ExternalInput"),
        "dect": ([D, TC], f32r, "ExternalInput"),
        "dec": ([TC, D], f32, "ExternalInput"),
        "wq": ([D, D], f32r, "ExternalInput"),
        "wkt": ([D, D], f32r, "ExternalInput"),
        "wvt": ([D, D], f32r, "ExternalInput"),
        "bqh": ([DH, H], f32r, "ExternalInput"),
        "ones": ([1, 128], f32r, "ExternalInput"),
        "kvc": ([DH, D], f32, "ExternalInput"),
        "gamma": ([D], f32, "ExternalInput"),
        "beta": ([D], f32, "ExternalInput"),
        "out": ([TC, D], f32, "ExternalOutput"),
    }
    d = {
        name: nc.dram_tensor(name, shape, dt, kind=kind).ap()
        for name, (shape, dt, kind) in specs.items()
    }

    with tile.TileContext(nc) as tc:
        _body(tc, nc, d, t_enc)
    nc.compile()
    return nc


def make_in_maps(inputs):
    """Shard full inputs into 8 per-core input maps."""
    enc_f = np.ascontiguousarray(np.asarray(inputs["encoder_seq"], dtype=np.float32))
    dec_f = np.ascontiguousarray(np.asarray(inputs["decoder_seq"], dtype=np.float32))
    wq = np.ascontiguousarray(np.asarray(inputs["Wq"], dtype=np.float32))
    wk = np.asarray(inputs["Wk"], dtype=np.float32)
    wv = np.asarray(inputs["Wv"], dtype=np.float32)
    bq = np.asarray(inputs["bq"], dtype=np.float32)
    bk = np.asarray(inputs["bk"], dtype=np.float32)
    bv = np.asarray(inputs["bv"], dtype=np.float32)
    gamma = np.ascontiguousarray(np.asarray(inputs["ln_gamma"], dtype=np.float32))
    beta = np.ascontiguousarray(np.asarray(inputs["ln_beta"], dtype=np.float32))

    wkt = np.ascontiguousarray(wk.T)
    wvt = np.ascontiguousarray(wv.T)
    bqh = np.ascontiguousarray(bq.reshape(H, DH).T)  # [DH, H]
    ones = np.ones((1, 128), dtype=np.float32)

    in_maps = []
    for c in range(8):
        b, p = divmod(c, 2)
        enc_b = np.ascontiguousarray(enc_f[b])  # [T_ENC, D]
        dec_h = np.ascontiguousarray(dec_f[b, p * TC:(p + 1) * TC])  # [TC, D]
        dect = np.ascontiguousarray(dec_h.T)  # [D, TC]

        # kv bias corrections (rank-1; exactly zero for zero biases):
        # kv_full[h] = k_h.T v_h with k = enc Wk.T + bk, v = enc Wv.T + bv
        #   = Wk_h G Wv_h.T + bk_h (x) sv_h + sk_h (x) bv_h + T bk_h (x) bv_h
        # where s = sum_t enc[t], sk_h = Wk_h s, sv_h = Wv_h s.
        kvc = np.zeros((DH, D), dtype=np.float32)
        if bk.any() or bv.any():
            s = enc_b.sum(axis=0)  # [D]
            sk = (wk @ s).reshape(H, DH)
            sv = (wv @ s).reshape(H, DH)
            bkh = bk.reshape(H, DH)
            bvh = bv.reshape(H, DH)
            for h in range(H):
                corr = (
                    np.outer(bkh[h], sv[h])
                    + np.outer(sk[h], bvh[h])
                    + T_ENC * np.outer(bkh[h], bvh[h])
                )
                kvc[:, h * DH:(h + 1) * DH] = corr
            kvc *= SCALE
        in_maps.append({
            "enc": enc_b,
            "dect": dect,
            "dec": dec_h,
            "wq": wq,
            "wkt": wkt,
            "wvt": wvt,
            "bqh": bqh,
            "ones": ones,
            "kvc": kvc,
            "gamma": gamma,
            "beta": beta,
        })
    return in_maps


def kernel(**inputs):
    if "nc" not in _CACHE:
        _CACHE["nc"] = build_program()
    nc = _CACHE["nc"]
    in_maps = make_in_maps(inputs)
    res = bass_utils.run_bass_kernel_spmd(nc, in_maps, core_ids=list(range(8)))
    out = np.empty((B, T, D), dtype=np.float32)
    for c in range(8):
        b, p = divmod(c, 2)
        out[b, p * TC:(p + 1) * TC] = res.results[c]["out"]
    return out


# revision 25
# speedup vs baseline: 1.3844x; 1.0336x over previous
"""Trainium2 Bass kernel for a softmax-free cross-attention block.

Math (per batch b):
  q  = dec @ Wq.T + bq                       [T, D]
  k  = enc @ Wk.T + bk ; v = enc @ Wv.T + bv [T, D]
  kv[h] = k_h.T @ v_h                        [dh, dh]  (contraction over T_enc)
  att   = scale * q_h @ kv[h]                [T, D]
  out   = LayerNorm(att + dec) * gamma + beta

Key algebraic restructuring (all O(n^3) work stays on device):
  kv[h] = Wk_h (enc.T enc) Wv_h.T  -- Gram matrix G replaces the K/V
  projections (one G serves all heads / both K and V), and
  att = dec @ WeffT with WeffT[:, h] = Wq_h.T kv[h] -- a single dense
  matmul replaces Q-projection + per-head attention apply.

Sharding: 8 cores = 4 batches x 2 decoder-halves. Each core computes G for
its full batch (duplicated within the pair -- no collectives) and the output
rows for its 1024 decoder tokens.

Bias handling: bq is applied exactly on-device (rank-1 matmul into the att
accumulation); bk/bv enter kv only through rank-1 correction terms which are
precomputed on host (O(D^2) work; exactly zero for the given inputs).

Matmuls run as float32r (tf32-like, ~1e-4 relative error), accumulation fp32.
"""

import numpy as np

import concourse.bass as bass
import concourse.mybir as mybir
import concourse.tile as tile
from concourse import bacc, bass_utils

D = 1024
H = 16
DH = 64
T_ENC = 2048  # encoder tokens per batch (full batch per core)
TC = 1024  # decoder tokens per core
NT = D // 128  # 8 tiles of 128 along any D-sized dim
NTE = T_ENC // 128  # 16 encoder token tiles
B = 4
T = 2048
SCALE = 1.0 / np.sqrt(DH)
LN_EPS = 1e-5

_CACHE = {}


def _body(tc, nc, d, t_enc):
    f32 = mybir.dt.float32
    f32r = mybir.dt.float32r
    AF = mybir.ActivationFunctionType
    OP = mybir.AluOpType
    nte = t_enc // 128

    ap = tc.alloc_tile_pool  # shorthand

    small = ap(name="small", bufs=1, side="left")
    psum_big = ap(name="psum_big", bufs=3, space="PSUM")
    psum_kv = ap(name="psum_kv", bufs=1, space="PSUM")
    ep = ap(name="ep", bufs=3, side="left")
    ep1 = ap(name="ep1", bufs=4, side="left")

    # kv [64, D] computed on partitions 0-63, then duplicated to 64-127 so the
    # Weff stage can pair it with natural-Wq row slices at base partition 0/64
    kv_sb = small.tile([128, D], f32r, tag="kv")
    attb_sb = small.tile([1, D], f32r, tag="attb")
    bq_sb = small.tile([DH, H], f32r, tag="bq")
    ones_sb = small.tile([1, 128], f32r, tag="ones")
    kvc_sb = small.tile([DH, D], f32, tag="kvc")
    gam_sb = small.tile([128, D], f32, tag="gam")
    bet_sb = small.tile([128, D], f32, tag="bet")
    eps_sb = small.tile([128, 1], f32, tag="eps")

    # ---- Phase G: G = enc.T @ enc  [D, D] ----------------------------------
    # Pool plan (left/right stacks so weight loads prefetch into fresh space):
    #  left:  g(32K) t1(32K) | wq(32K after enc freed)
    #  right: wk(32K) enc(64K->freed) | wv(32K) | weff(32K) dect(32K)
    g_pool = ap(name="gp", bufs=1, side="left")
    g_sb = g_pool.tile([128, NT, D], f32r, tag="g")
    t1_pool = ap(name="t1p", bufs=1, side="left")
    t1_sb = t1_pool.tile([128, NT, D], f32r, tag="t1")
    wk_pool = ap(name="wkp", bufs=1, side="right")
    wk_sb = wk_pool.tile([128, NT, D], f32r, tag="wk")
    enc_pool = ap(name="encp", bufs=1, side="right")
    enc_sb = enc_pool.tile([128, nte, D], f32r, tag="enc")
    for t in range(nte):
        nc.sync.dma_start(out=enc_sb[:, t, :], in_=d["enc"][t * 128:(t + 1) * 128, :])
    for i in range(NT):
        nc.sync.dma_start(out=wk_sb[:, i, :], in_=d["wkt"][i * 128:(i + 1) * 128, :])
    nc.sync.dma_start(out=bq_sb, in_=d["bqh"])
    nc.sync.dma_start(out=ones_sb, in_=d["ones"])
    nc.sync.dma_start(out=kvc_sb, in_=d["kvc"])
    gam = d["gamma"]
    bet = d["beta"]
    nc.sync.dma_start(
        out=gam_sb,
        in_=bass.AP(tensor=gam.tensor, offset=gam.offset, ap=[[0, 128]] + gam.ap),
    )
    nc.sync.dma_start(
        out=bet_sb,
        in_=bass.AP(tensor=bet.tensor, offset=bet.offset, ap=[[0, 128]] + bet.ap),
    )
    nc.vector.memset(eps_sb, LN_EPS)
    # two sweeps over encoder halves so matmuls start after ~half the encoder
    # DMA has landed (psum can't hold 8 i-tiles at once, SBUF accumulates)
    half = nte // 2
    for sweep in range(2):
        for i in range(NT):
            pg = psum_big.tile([128, D], f32, tag="pbig")
            for tt in range(half):
                t = sweep * half + tt
                st, sp = tt == 0, tt == half - 1
                lhs = enc_sb[:, t, i * 128:(i + 1) * 128]
                nc.tensor.matmul(pg[:, 0:512], lhs, enc_sb[:, t, 0:512], start=st, stop=sp)
                nc.tensor.matmul(pg[:, 512:1024], lhs, enc_sb[:, t, 512:1024], start=st, stop=sp)
            if sweep == 0:
                nc.scalar.copy(out=g_sb[:, i, :], in_=pg[:, :])
            else:
                nc.vector.tensor_add(out=g_sb[:, i, :], in0=g_sb[:, i, :], in1=pg[:, :])
    enc_pool.release()

    # wq into enc's old space (loads overlap T1t), wv into fresh right space
    wq_pool = ap(name="wqp", bufs=1, side="left")
    wq_sb = wq_pool.tile([128, NT, D], f32r, tag="wqn")
    for r in range(NT):
        nc.sync.dma_start(out=wq_sb[:, r, :], in_=d["wq"][r * 128:(r + 1) * 128, :])
    wv_pool = ap(name="wvp", bufs=1, side="right")
    wv_sb = wv_pool.tile([128, NT, D], f32r, tag="wv")
    for j in range(NT):
        nc.sync.dma_start(out=wv_sb[:, j, :], in_=d["wvt"][j * 128:(j + 1) * 128, :])

    # ---- Phase T1t: T1t = G @ Wk.T  ( = (Wk G).T )  [D(j), D(hd)] ----------
    for j in range(NT):
        pt = psum_big.tile([128, D], f32, tag="pbig")
        for i in range(NT):
            st, sp = i == 0, i == NT - 1
            lhs = g_sb[:, i, j * 128:(j + 1) * 128]
            nc.tensor.matmul(pt[:, 0:512], lhs, wk_sb[:, i, 0:512], start=st, stop=sp)
            nc.tensor.matmul(pt[:, 512:1024], lhs, wk_sb[:, i, 512:1024], start=st, stop=sp)
        nc.scalar.copy(out=t1_sb[:, j, :], in_=pt[:, :])

    # ---- Phase kv: kv[h] = T1t_h.T @ WvT_h  [64, D] on partitions 0-63 -----
    pkv = psum_kv.tile([DH, D], f32, tag="pkv")
    for h in range(H):
        hs = slice(h * DH, (h + 1) * DH)
        for j in range(NT):
            st = h % 8 == 0 and j == 0
            sp = h % 8 == 7 and j == NT - 1
            nc.tensor.matmul(pkv[:, hs], t1_sb[:, j, hs], wv_sb[:, j, hs], start=st, stop=sp)
    # fold the attention scale into kv, then add host-precomputed bias
    # corrections (pre-scaled on host; zeros when biases are zero)
    nc.vector.tensor_scalar(
        out=kv_sb[0:DH, :], in0=pkv[:, :], scalar1=SCALE, scalar2=None, op0=OP.mult,
    )
    nc.vector.tensor_add(out=kv_sb[0:DH, :], in0=kv_sb[0:DH, :], in1=kvc_sb)
    # duplicate to partitions 64-127 for the Weff stage's odd-head row groups
    nc.sync.dma_start(out=kv_sb[DH:2 * DH, :], in_=kv_sb[0:DH, :])
    wv_pool.release()
    wk_pool.release()

    # ---- attb = bq_h.T @ kv[h] (row vector added to every att row) ---------
    pab = psum_kv.tile([DH, D], f32, tag="pkv")
    for h in range(H):
        hs = slice(h * DH, (h + 1) * DH)
        st, sp = h % 8 == 0, h % 8 == 7
        nc.tensor.matmul(
            pab[0:1, hs], bq_sb[:, h:h + 1], kv_sb[0:DH, hs], start=st, stop=sp,
        )
    nc.scalar.copy(out=attb_sb, in_=pab[0:1, :])

    # ---- Phase Weff: WeffT[i, he] = sum_d Wq[hd, i] kv[h][d, e] ------------
    # consecutive heads alternate PE row-groups (partition offset 0/64) so their
    # weight loads and matmuls overlap in the array
    weff_pool = ap(name="weffp", bufs=1, side="right")
    weff_sb = weff_pool.tile([128, NT, D], f32r, tag="weff")
    dt_pool = ap(name="dtp", bufs=1, side="right")
    dt_sb = dt_pool.tile([128, NT, TC], f32r, tag="dect")
    for i in range(NT):
        nc.sync.dma_start(out=dt_sb[:, i, :], in_=d["dect"][i * 128:(i + 1) * 128, :])
    for i in range(NT):
        pw = psum_big.tile([128, D], f32, tag="pbig")
        # group heads by partition base: interleaving base-0/base-64 fp32r
        # matmuls per-MM hangs the fp32 weight-load path on HW
        order = [h for h in range(H) if h % 2 == 0] + [h for h in range(H) if h % 2 == 1]
        for idx, h in enumerate(order):
            hs = slice(h * DH, (h + 1) * DH)
            po = (h % 2) * DH
            bank = h // 8
            st = all(order[k] // 8 != bank for k in range(idx))
            sp = all(order[k] // 8 != bank for k in range(idx + 1, H))
            lhs = wq_sb[po:po + DH, h // 2, i * 128:(i + 1) * 128]
            nc.tensor.matmul(pw[:, hs], lhs, kv_sb[po:po + DH, hs], start=st, stop=sp)
        nc.scalar.copy(out=weff_sb[:, i, :], in_=pw[:, :])

    # ---- Phase att + residual + LayerNorm ----------------------------------
    for t in range(TC // 128):
        pa = psum_big.tile([128, D], f32, tag="pbig")
        dct = ep.tile([128, D], f32, tag="dc")
        nc.sync.dma_start(out=dct, in_=d["dec"][t * 128:(t + 1) * 128, :])
        for i in range(NT):
            st = i == 0
            lhs = dt_sb[:, i, t * 128:(t + 1) * 128]
            nc.tensor.matmul(pa[:, 0:512], lhs, weff_sb[:, i, 0:512], start=st, stop=False)
            nc.tensor.matmul(pa[:, 512:1024], lhs, weff_sb[:, i, 512:1024], start=st, stop=False)
        # q-bias contribution: broadcast attb row into every token row
        nc.tensor.matmul(pa[:, 0:512], ones_sb, attb_sb[:, 0:512], start=False, stop=True)
        nc.tensor.matmul(pa[:, 512:1024], ones_sb, attb_sb[:, 512:1024], start=False, stop=True)

        # residual add straight from PSUM (scale already folded into kv)
        x = ep.tile([128, D], f32, tag="x")
        nc.vector.tensor_add(out=x, in0=pa[:, :], in1=dct)
        stats = ep1.tile([128, 2, 6], f32, tag="st")
        nc.vector.bn_stats(out=stats[:, 0, :], in_=x[:, 0:512])
        nc.vector.bn_stats(out=stats[:, 1, :], in_=x[:, 512:1024])
        mv = ep1.tile([128, 2], f32, tag="mv")
        nc.vector.bn_aggr(out=mv, in_=stats)
        rstd = ep1.tile([128, 1], f32, tag="rs")
        nc.scalar.activation(out=rstd, in_=mv[:, 1:2], func=AF.Sqrt, bias=eps_sb, scale=1.0)
        nc.vector.reciprocal(out=rstd, in_=rstd)
        nc.vector.tensor_scalar(
            out=x, in0=x, scalar1=mv[:, 0:1], scalar2=rstd,
            op0=OP.subtract, op1=OP.mult,
        )
        nc.vector.tensor_mul(out=x, in0=x, in1=gam_sb)
        nc.gpsimd.tensor_add(out=x, in0=x, in1=bet_sb)
        nc.sync.dma_start(out=d["out"][t * 128:(t + 1) * 128, :], in_=x)
    dt_pool.release()
    weff_pool.release()
    wq_pool.release()
    t1_pool.release()
    g_pool.release()

    ep1.release()
    ep.release()
    psum_kv.release()
    psum_big.release()
    small.release()


def build_program(t_enc=T_ENC):
    nc = bacc.Bacc(
        "TRN2",
        target_bir_lowering=False,
        debug=False,
        enable_asserts=False,
        num_devices=8,
    )
    f32 = mybir.dt.float32
    f32r = mybir.dt.float32r
    specs = {
        "enc": ([t_enc, D], f32r, "ExternalInput"),
        "dect": ([D, TC], f32r, "ExternalInput"),
        "dec": ([TC, D], f32, "ExternalInput"),
        "wq": ([D, D], f32r, "ExternalInput"),
        "wkt": ([D, D], f32r, "ExternalInput"),
        "wvt": ([D, D], f32r, "ExternalInput"),
        "bqh": ([DH, H], f32r, "ExternalInput"),
        "ones": ([1, 128], f32r, "ExternalInput"),
        "kvc": ([DH, D], f32, "ExternalInput"),
        "gamma": ([D], f32, "ExternalInput"),
        "beta": ([D], f32, "ExternalInput"),
        "out": ([TC, D], f32, "ExternalOutput"),
    }
    d = {
        name: nc.dram_tensor(name, shape, dt, kind=kind).ap()
        for name, (shape, dt, kind) in specs.items()
    }

    with tile.TileContext(nc) as tc:
        _body(tc, nc, d, t_enc)
    nc.compile()
    return nc


def make_in_maps(inputs):
    """Shard full inputs into 8 per-core input maps."""
    enc_f = np.ascontiguousarray(np.asarray(inputs["encoder_seq"], dtype=np.float32))
    dec_f = np.ascontiguousarray(np.asarray(inputs["decoder_seq"], dtype=np.float32))
    wq = np.ascontiguousarray(np.asarray(inputs["Wq"], dtype=np.float32))
    wk = np.asarray(inputs["Wk"], dtype=np.float32)
    wv = np.asarray(inputs["Wv"], dtype=np.float32)
    bq = np.asarray(inputs["bq"], dtype=np.float32)
    bk = np.asarray(inputs["bk"], dtype=np.float32)
    bv = np.asarray(inputs["bv"], dtype=np.float32)
    gamma = np.ascontiguousarray(np.asarray(inputs["ln_gamma"], dtype=np.float32))
    beta = np.ascontiguousarray(np.asarray(inputs["ln_beta"], dtype=np.float32))

    wkt = np.ascontiguousarray(wk.T)
    wvt = np.ascontiguousarray(wv.T)
    bqh = np.ascontiguousarray(bq.reshape(H, DH).T)  # [DH, H]
    ones = np.ones((1, 128), dtype=np.float32)

    in_maps = []
    for c in range(8):
        b, p = divmod(c, 2)
        enc_b = np.ascontiguousarray(enc_f[b])  # [T_ENC, D]
        dec_h = np.ascontiguousarray(dec_f[b, p * TC:(p + 1) * TC])  # [TC, D]
        dect = np.ascontiguousarray(dec_h.T)  # [D, TC]

        # kv bias corrections (rank-1; exactly zero for zero biases):
        # kv_full[h] = k_h.T v_h with k = enc Wk.T + bk, v = enc Wv.T + bv
        #   = Wk_h G Wv_h.T + bk_h (x) sv_h + sk_h (x) bv_h + T bk_h (x) bv_h
        # where s = sum_t enc[t], sk_h = Wk_h s, sv_h = Wv_h s.
        kvc = np.zeros((DH, D), dtype=np.float32)
        if bk.any() or bv.any():
            s = enc_b.sum(axis=0)  # [D]
            sk = (wk @ s).reshape(H, DH)
            sv = (wv @ s).reshape(H, DH)
            bkh = bk.reshape(H, DH)
            bvh = bv.reshape(H, DH)
            for h in range(H):
                corr = (
                    np.outer(bkh[h], sv[h])
                    + np.outer(sk[h], bvh[h])
                    + T_ENC * np.outer(bkh[h], bvh[h])
                )
                kvc[:, h * DH:(h + 1) * DH] = corr
            kvc *= SCALE
        in_maps.append({
            "enc": enc_b,
            "dect": dect,
            "dec": dec_h,
            "wq": wq,
            "wkt": wkt,
            "wvt": wvt,
            "bqh": bqh,
            "ones": ones,
            "kvc": kvc,
            "gamma": gamma,
            "beta": beta,
        })
    return in_maps


def kernel(**inputs):
    if "nc" not in _CACHE:
        _CACHE["nc"] = build_program()
    nc = _CACHE["nc"]
    in_maps = make_in_maps(inputs)
    res = bass_utils.run_bass_kernel_spmd(nc, in_maps, core_ids=list(range(8)))
    out = np.empty((B, T, D), dtype=np.float32)
    for c in range(8):
        b, p = divmod(c, 2)
        out[b, p * TC:(p + 1) * TC] = res.results[c]["out"]
    return out


# revision 28
# speedup vs baseline: 1.5291x; 1.1045x over previous
"""Trainium2 Bass kernel for a softmax-free cross-attention block.

Math (per batch b):
  q  = dec @ Wq.T + bq                       [T, D]
  k  = enc @ Wk.T + bk ; v = enc @ Wv.T + bv [T, D]
  kv[h] = k_h.T @ v_h                        [dh, dh]  (contraction over T_enc)
  att   = scale * q_h @ kv[h]                [T, D]
  out   = LayerNorm(att + dec) * gamma + beta

Key algebraic restructuring (all O(n^3) work stays on device):
  kv[h] = Wk_h (enc.T enc) Wv_h.T  -- Gram matrix G replaces the K/V
  projections (one G serves all heads / both K and V), and
  att = dec @ WeffT with WeffT[:, h] = Wq_h.T kv[h] -- a single dense
  matmul replaces Q-projection + per-head attention apply.

Sharding: 8 cores = 4 batches x 2 decoder-halves. Each core computes G for
its full batch (duplicated within the pair -- no collectives) and the output
rows for its 1024 decoder tokens.

Bias handling: bq is applied exactly on-device (rank-1 matmul into the att
accumulation); bk/bv enter kv only through rank-1 correction terms which are
precomputed on host (O(D^2) work; exactly zero for the given inputs).

Matmuls run as float32r (tf32-like, ~1e-4 relative error), accumulation fp32.
"""

import numpy as np

import concourse.bass as bass
import concourse.mybir as mybir
import concourse.tile as tile
from concourse import bacc, bass_utils

D = 1024
H = 16
DH = 64
T_ENC = 2048  # encoder tokens per batch (full batch per core)
TC = 1024  # decoder tokens per core
NT = D // 128  # 8 tiles of 128 along any D-sized dim
NTE = T_ENC // 128  # 16 encoder token tiles
B = 4
T = 2048
SCALE = 1.0 / np.sqrt(DH)
LN_EPS = 1e-5

_CACHE = {}


def _body(tc, nc, d, t_enc):
    f32 = mybir.dt.float32
    f32r = mybir.dt.float32r
    AF = mybir.ActivationFunctionType
    OP = mybir.AluOpType
    nte = t_enc // 128

    ap = tc.alloc_tile_pool  # shorthand

    small = ap(name="small", bufs=1, side="left")
    psum_big = ap(name="psum_big", bufs=2, space="PSUM")
    psum_tr = ap(name="psum_tr", bufs=2, space="PSUM")
    psum_kv = ap(name="psum_kv", bufs=1, space="PSUM")
    ep = ap(name="ep", bufs=3, side="left")
    ep1 = ap(name="ep1", bufs=4, side="left")

    # kv [64, D] computed on partitions 0-63, then duplicated to 64-127 so the
    # Weff stage can pair it with natural-Wq row slices at base partition 0/64
    kv_sb = small.tile([128, D], f32r, tag="kv")
    attb_sb = small.tile([1, D], f32r, tag="attb")
    bq_sb = small.tile([DH, H], f32r, tag="bq")
    ones_sb = small.tile([1, 128], f32r, tag="ones")
    kvc_sb = small.tile([DH, D], f32, tag="kvc")
    gam_sb = small.tile([128, D], f32, tag="gam")
    bet_sb = small.tile([128, D], f32, tag="bet")
    eps_sb = small.tile([128, 1], f32, tag="eps")
    ident_sb = small.tile([128, 128], f32r, tag="ident")

    # ---- Phase G: G = enc.T @ enc  [D, D] ----------------------------------
    # Pool plan (left/right stacks so weight loads prefetch into fresh space):
    #  left:  g(32K) t1(32K) | wq(32K after enc freed)
    #  right: wk(32K) enc(64K->freed) | wv(32K) | weff(32K) dect(32K)
    g_pool = ap(name="gp", bufs=1, side="left")
    g_sb = g_pool.tile([128, NT, D], f32r, tag="g")
    t1_pool = ap(name="t1p", bufs=1, side="left")
    t1_sb = t1_pool.tile([128, NT, D], f32r, tag="t1")
    wk_pool = ap(name="wkp", bufs=1, side="right")
    wk_sb = wk_pool.tile([128, NT, D], f32r, tag="wk")
    enc_pool = ap(name="encp", bufs=1, side="right")
    enc_sb = enc_pool.tile([128, nte, D], f32r, tag="enc")
    for t in range(nte):
        nc.sync.dma_start(out=enc_sb[:, t, :], in_=d["enc"][t * 128:(t + 1) * 128, :])
    for i in range(NT):
        nc.sync.dma_start(out=wk_sb[:, i, :], in_=d["wkt"][i * 128:(i + 1) * 128, :])
    nc.sync.dma_start(out=ident_sb, in_=d["ident"])
    nc.sync.dma_start(out=bq_sb, in_=d["bqh"])
    nc.sync.dma_start(out=ones_sb, in_=d["ones"])
    nc.sync.dma_start(out=kvc_sb, in_=d["kvc"])
    gam = d["gamma"]
    bet = d["beta"]
    nc.sync.dma_start(
        out=gam_sb,
        in_=bass.AP(tensor=gam.tensor, offset=gam.offset, ap=[[0, 128]] + gam.ap),
    )
    nc.sync.dma_start(
        out=bet_sb,
        in_=bass.AP(tensor=bet.tensor, offset=bet.offset, ap=[[0, 128]] + bet.ap),
    )
    nc.vector.memset(eps_sb, LN_EPS)
    # two sweeps over encoder halves so matmuls start after ~half the encoder
    # DMA has landed (psum can't hold 8 i-tiles at once, SBUF accumulates).
    # G is symmetric: row-tile i only computes columns [LO[i], D) directly
    # (chunk cuts chosen so every fp32r matmul keeps free dim >= 256); the
    # missing lower blocks are filled by PE transposes of the upper ones.
    LO = [0, 128, 256, 256, 512, 512, 768, 768]
    half = nte // 2
    for sweep in range(2):
        for i in range(NT):
            lo = LO[i]
            chunks = [(lo, 512), (512, D)] if lo < 512 else [(lo, D)]
            pg = psum_big.tile([128, D], f32, tag="pbig")
            for tt in range(half):
                t = sweep * half + tt
                st, sp = tt == 0, tt == half - 1
                lhs = enc_sb[:, t, i * 128:(i + 1) * 128]
                for c0, c1 in chunks:
                    nc.tensor.matmul(pg[:, c0:c1], lhs, enc_sb[:, t, c0:c1], start=st, stop=sp)
            if sweep == 0:
                nc.scalar.copy(out=g_sb[:, i, lo:D], in_=pg[:, lo:D])
            else:
                nc.vector.tensor_add(
                    out=g_sb[:, i, lo:D], in0=g_sb[:, i, lo:D], in1=pg[:, lo:D]
                )
    enc_pool.release()
    # fill lower-triangle blocks: G[i][:, j] = G[j][:, i].T
    for i in range(NT):
        for j in range(LO[i] // 128):
            tp = psum_tr.tile([128, 128], f32r, tag="ptr")
            nc.tensor.transpose(tp[:, :], g_sb[:, j, i * 128:(i + 1) * 128], ident_sb[:, :])
            nc.scalar.copy(out=g_sb[:, i, j * 128:(j + 1) * 128], in_=tp[:, :])

    # wq into enc's old space (loads overlap T1t), wv into fresh right space
    wq_pool = ap(name="wqp", bufs=1, side="left")
    wq_sb = wq_pool.tile([128, NT, D], f32r, tag="wqn")
    for r in range(NT):
        nc.sync.dma_start(out=wq_sb[:, r, :], in_=d["wq"][r * 128:(r + 1) * 128, :])
    wv_pool = ap(name="wvp", bufs=1, side="right")
    wv_sb = wv_pool.tile([128, NT, D], f32r, tag="wv")
    for j in range(NT):
        nc.sync.dma_start(out=wv_sb[:, j, :], in_=d["wvt"][j * 128:(j + 1) * 128, :])

    # ---- Phase T1t: T1t = G @ Wk.T  ( = (Wk G).T )  [D(j), D(hd)] ----------
    for j in range(NT):
        pt = psum_big.tile([128, D], f32, tag="pbig")
        for i in range(NT):
            st, sp = i == 0, i == NT - 1
            lhs = g_sb[:, i, j * 128:(j + 1) * 128]
            nc.tensor.matmul(pt[:, 0:512], lhs, wk_sb[:, i, 0:512], start=st, stop=sp)
            nc.tensor.matmul(pt[:, 512:1024], lhs, wk_sb[:, i, 512:1024], start=st, stop=sp)
        nc.scalar.copy(out=t1_sb[:, j, :], in_=pt[:, :])

    # ---- Phase kv: kv[h] = T1t_h.T @ WvT_h  [64, D] on partitions 0-63 -----
    pkv = psum_kv.tile([DH, D], f32, tag="pkv")
    for h in range(H):
        hs = slice(h * DH, (h + 1) * DH)
        for j in range(NT):
            st = h % 8 == 0 and j == 0
            sp = h % 8 == 7 and j == NT - 1
            nc.tensor.matmul(pkv[:, hs], t1_sb[:, j, hs], wv_sb[:, j, hs], start=st, stop=sp)
    # fold the attention scale into kv, then add host-precomputed bias
    # corrections (pre-scaled on host; zeros when biases are zero)
    nc.vector.tensor_scalar(
        out=kv_sb[0:DH, :], in0=pkv[:, :], scalar1=SCALE, scalar2=None, op0=OP.mult,
    )
    nc.vector.tensor_add(out=kv_sb[0:DH, :], in0=kv_sb[0:DH, :], in1=kvc_sb)
    # duplicate to partitions 64-127 for the Weff stage's odd-head row groups
    nc.sync.dma_start(out=kv_sb[DH:2 * DH, :], in_=kv_sb[0:DH, :])
    wv_pool.release()
    wk_pool.release()

    # ---- attb = bq_h.T @ kv[h] (row vector added to every att row) ---------
    pab = psum_kv.tile([DH, D], f32, tag="pkv")
    for h in range(H):
        hs = slice(h * DH, (h + 1) * DH)
        st, sp = h % 8 == 0, h % 8 == 7
        nc.tensor.matmul(
            pab[0:1, hs], bq_sb[:, h:h + 1], kv_sb[0:DH, hs], start=st, stop=sp,
        )
    nc.scalar.copy(out=attb_sb, in_=pab[0:1, :])

    # ---- Phase Weff: WeffT[i, he] = sum_d Wq[hd, i] kv[h][d, e] ------------
    # consecutive heads alternate PE row-groups (partition offset 0/64) so their
    # weight loads and matmuls overlap in the array
    weff_pool = ap(name="weffp", bufs=1, side="right")
    weff_sb = weff_pool.tile([128, NT, D], f32r, tag="weff")
    dt_pool = ap(name="dtp", bufs=1, side="right")
    dt_sb = dt_pool.tile([128, NT, TC], f32r, tag="dect")
    for i in range(NT):
        nc.sync.dma_start(out=dt_sb[:, i, :], in_=d["dect"][i * 128:(i + 1) * 128, :])
    for i in range(NT):
        pw = psum_big.tile([128, D], f32, tag="pbig")
        # group heads by partition base: interleaving base-0/base-64 fp32r
        # matmuls per-MM hangs the fp32 weight-load path on HW
        order = [h for h in range(H) if h % 2 == 0] + [h for h in range(H) if h % 2 == 1]
        for idx, h in enumerate(order):
            hs = slice(h * DH, (h + 1) * DH)
            po = (h % 2) * DH
            bank = h // 8
            st = all(order[k] // 8 != bank for k in range(idx))
            sp = all(order[k] // 8 != bank for k in range(idx + 1, H))
            lhs = wq_sb[po:po + DH, h // 2, i * 128:(i + 1) * 128]
            nc.tensor.matmul(pw[:, hs], lhs, kv_sb[po:po + DH, hs], start=st, stop=sp)
        nc.scalar.copy(out=weff_sb[:, i, :], in_=pw[:, :])

    # ---- Phase att + residual + LayerNorm ----------------------------------
    for t in range(TC // 128):
        pa = psum_big.tile([128, D], f32, tag="pbig")
        dct = ep.tile([128, D], f32, tag="dc")
        nc.sync.dma_start(out=dct, in_=d["dec"][t * 128:(t + 1) * 128, :])
        for i in range(NT):
            st = i == 0
            lhs = dt_sb[:, i, t * 128:(t + 1) * 128]
            nc.tensor.matmul(pa[:, 0:512], lhs, weff_sb[:, i, 0:512], start=st, stop=False)
            nc.tensor.matmul(pa[:, 512:1024], lhs, weff_sb[:, i, 512:1024], start=st, stop=False)
        # q-bias contribution: broadcast attb row into every token row
        nc.tensor.matmul(pa[:, 0:512], ones_sb, attb_sb[:, 0:512], start=False, stop=True)
        nc.tensor.matmul(pa[:, 512:1024], ones_sb, attb_sb[:, 512:1024], start=False, stop=True)

        # residual add straight from PSUM (scale already folded into kv)
        x = ep.tile([128, D], f32, tag="x")
        nc.vector.tensor_add(out=x, in0=pa[:, :], in1=dct)
        stats = ep1.tile([128, 2, 6], f32, tag="st")
        nc.vector.bn_stats(out=stats[:, 0, :], in_=x[:, 0:512])
        nc.vector.bn_stats(out=stats[:, 1, :], in_=x[:, 512:1024])
        mv = ep1.tile([128, 2], f32, tag="mv")
        nc.vector.bn_aggr(out=mv, in_=stats)
        rstd = ep1.tile([128, 1], f32, tag="rs")
        nc.scalar.activation(out=rstd, in_=mv[:, 1:2], func=AF.Sqrt, bias=eps_sb, scale=1.0)
        nc.vector.reciprocal(out=rstd, in_=rstd)
        nc.vector.tensor_scalar(
            out=x, in0=x, scalar1=mv[:, 0:1], scalar2=rstd,
            op0=OP.subtract, op1=OP.mult,
        )
        nc.vector.tensor_mul(out=x, in0=x, in1=gam_sb)
        nc.gpsimd.tensor_add(out=x, in0=x, in1=bet_sb)
        nc.sync.dma_start(out=d["out"][t * 128:(t + 1) * 128, :], in_=x)
    dt_pool.release()
    weff_pool.release()
    wq_pool.release()
    t1_pool.release()
    g_pool.release()

    ep1.release()
    ep.release()
    psum_kv.release()
    psum_tr.release()
    psum_big.release()
    small.release()


def build_program(t_enc=T_ENC):
    nc = bacc.Bacc(
        "TRN2",
        target_bir_lowering=False,
        debug=False,
        enable_asserts=False,
        num_devices=8,
    )
    f32 = mybir.dt.float32
    f32r = mybir.dt.float32r
    specs = {
        "enc": ([t_enc, D], f32r, "ExternalInput"),
        "dect": ([D, TC], f32r, "ExternalInput"),
        "dec": ([TC, D], f32, "ExternalInput"),
        "wq": ([D, D], f32r, "ExternalInput"),
        "wkt": ([D, D], f32r, "ExternalInput"),
        "wvt": ([D, D], f32r, "ExternalInput"),
        "bqh": ([DH, H], f32r, "ExternalInput"),
        "ones": ([1, 128], f32r, "ExternalInput"),
        "ident": ([128, 128], f32r, "ExternalInput"),
        "kvc": ([DH, D], f32, "ExternalInput"),
        "gamma": ([D], f32, "ExternalInput"),
        "beta": ([D], f32, "ExternalInput"),
        "out": ([TC, D], f32, "ExternalOutput"),
    }
    d = {
        name: nc.dram_tensor(name, shape, dt, kind=kind).ap()
        for name, (shape, dt, kind) in specs.items()
    }

    with tile.TileContext(nc) as tc:
        _body(tc, nc, d, t_enc)
    nc.compile()
    return nc


def make_in_maps(inputs):
    """Shard full inputs into 8 per-core input maps."""
    enc_f = np.ascontiguousarray(np.asarray(inputs["encoder_seq"], dtype=np.float32))
    dec_f = np.ascontiguousarray(np.asarray(inputs["decoder_seq"], dtype=np.float32))
    wq = np.ascontiguousarray(np.asarray(inputs["Wq"], dtype=np.float32))
    wk = np.asarray(inputs["Wk"], dtype=np.float32)
    wv = np.asarray(inputs["Wv"], dtype=np.float32)
    bq = np.asarray(inputs["bq"], dtype=np.float32)
    bk = np.asarray(inputs["bk"], dtype=np.float32)
    bv = np.asarray(inputs["bv"], dtype=np.float32)
    gamma = np.ascontiguousarray(np.asarray(inputs["ln_gamma"], dtype=np.float32))
    beta = np.ascontiguousarray(np.asarray(inputs["ln_beta"], dtype=np.float32))

    wkt = np.ascontiguousarray(wk.T)
    wvt = np.ascontiguousarray(wv.T)
    bqh = np.ascontiguousarray(bq.reshape(H, DH).T)  # [DH, H]
    ones = np.ones((1, 128), dtype=np.float32)
    ident = np.eye(128, dtype=np.float32)

    in_maps = []
    for c in range(8):
        b, p = divmod(c, 2)
        enc_b = np.ascontiguousarray(enc_f[b])  # [T_ENC, D]
        dec_h = np.ascontiguousarray(dec_f[b, p * TC:(p + 1) * TC])  # [TC, D]
        dect = np.ascontiguousarray(dec_h.T)  # [D, TC]

        # kv bias corrections (rank-1; exactly zero for zero biases):
        # kv_full[h] = k_h.T v_h with k = enc Wk.T + bk, v = enc Wv.T + bv
        #   = Wk_h G Wv_h.T + bk_h (x) sv_h + sk_h (x) bv_h + T bk_h (x) bv_h
        # where s = sum_t enc[t], sk_h = Wk_h s, sv_h = Wv_h s.
        kvc = np.zeros((DH, D), dtype=np.float32)
        if bk.any() or bv.any():
            s = enc_b.sum(axis=0)  # [D]
            sk = (wk @ s).reshape(H, DH)
            sv = (wv @ s).reshape(H, DH)
            bkh = bk.reshape(H, DH)
            bvh = bv.reshape(H, DH)
            for h in range(H):
                corr = (
                    np.outer(bkh[h], sv[h])
                    + np.outer(sk[h], bvh[h])
                    + T_ENC * np.outer(bkh[h], bvh[h])
                )
                kvc[:, h * DH:(h + 1) * DH] = corr
            kvc *= SCALE
        in_maps.append({
            "enc": enc_b,
            "dect": dect,
            "dec": dec_h,
            "wq": wq,
            "wkt": wkt,
            "wvt": wvt,
            "bqh": bqh,
            "ones": ones,
            "ident": ident,
            "kvc": kvc,
            "gamma": gamma,
            "beta": beta,
        })
    return in_maps


def kernel(**inputs):
    if "nc" not in _CACHE:
        _CACHE["nc"] = build_program()
    nc = _CACHE["nc"]
    in_maps = make_in_maps(inputs)
    res = bass_utils.run_bass_kernel_spmd(nc, in_maps, core_ids=list(range(8)))
    out = np.empty((B, T, D), dtype=np.float32)
    for c in range(8):
        b, p = divmod(c, 2)
        out[b, p * TC:(p + 1) * TC] = res.results[c]["out"]
    return out
